# revision 6
# baseline (speedup 1.0000x reference)
"""AttentionBlock v3: fp8-DoubleRow GEMMs, GroupNorm folded into weights.

Changes vs v2 (which hit a 30us front dead-zone):
- x8 casts on ScalarE (GpSimd CAST was ~4x slower); all bn_stats on DVE (NBN=8).
- x DMA first, weights after (weights only needed ~when stats complete).
- Leaner stats->a,b chain (ind16 carries 1/16, Rsqrt, no 4096 scaling).
- PSUM: pp[128,2C]x2 + ps5[128,512]x3 + lgall (4 heads in one bank) = 8 banks.
- Phase C epilogues alternate ACT/DVE; Phase D epilogue on DVE with 3-deep ring.
"""

import os
import numpy as np
import ml_dtypes
from contextlib import ExitStack

import concourse.bass as bass
import concourse.bacc as bacc
import concourse.tile as tile
from concourse import mybir
from concourse.bass_utils import run_bass_kernel_spmd

F32 = mybir.dt.float32
BF16 = mybir.dt.bfloat16
F8 = mybir.dt.float8e4
DRM = mybir.MatmulPerfMode.DoubleRow
AX = mybir.AxisListType
OP = mybir.AluOpType
AF = mybir.ActivationFunctionType

B, C, H, W = 8, 512, 64, 64
HEADS, GROUPS, EPS = 4, 32, 1e-5
N = H * W
D = C // HEADS
NT = C // 128
NCH = N // 128
KCH = N // 512
WS = 32.0
SCALE = float(D) ** -0.5
SCALE_L = SCALE / (WS * WS)
SPINS = 106


def build_kernel(with_projb: bool) -> bass.Bass:
    nc = bacc.Bacc("TRN2")
    x_ext = nc.declare_dram_parameter("x", [NT, 128, N], F32, isOutput=False)
    w32_ext = nc.declare_dram_parameter("w8h", [2, 128, 2, 3 * C], F8, isOutput=False)
    p8_ext = nc.declare_dram_parameter("p8", [2, 128, 2, C], F8, isOutput=False)
    qkvb_ext = nc.declare_dram_parameter("qkv_b32", [3 * C], F32, isOutput=False)
    projb_ext = nc.declare_dram_parameter("proj_b", [C], F32, isOutput=False)
    gnw_ext = nc.declare_dram_parameter("gn_w", [C], F32, isOutput=False)
    gnb_ext = nc.declare_dram_parameter("gn_b", [C], F32, isOutput=False)
    ident_ext = nc.declare_dram_parameter("ident", [128, 128], BF16, isOutput=False)
    ind_ext = nc.declare_dram_parameter("ind16", [128, 8], F32, isOutput=False)
    indT_ext = nc.declare_dram_parameter("ind16T", [8, 128], F32, isOutput=False)
    out_ext = nc.declare_dram_parameter("out", [NT, 128, N], F32, isOutput=True)
    vbs_ext = nc.declare_dram_parameter("vbs", [1, C], F32, isOutput=True)

    with tile.TileContext(nc) as tc, ExitStack() as ctx:
        singles = ctx.enter_context(tc.tile_pool(name="singles", bufs=1))
        smalls = ctx.enter_context(tc.tile_pool(name="smalls", bufs=2))
        xres = ctx.enter_context(tc.tile_pool(name="xres", bufs=1))
        vpool = ctx.enter_context(tc.tile_pool(name="vpool", bufs=1))
        qkring = ctx.enter_context(tc.tile_pool(name="qkring", bufs=5))
        otring = ctx.enter_context(tc.tile_pool(name="otring", bufs=3))
        psum = ctx.enter_context(tc.tile_pool(name="psum", bufs=1, space="PSUM"))

        # ---- PE warm-up spin from a memset tile ----
        spin_rhs = singles.tile([128, 512], BF16, tag="spin_rhs", name="spin_rhs")
        nc.vector.memset(spin_rhs, 1.0)
        spin_ps = psum.tile([128, 512], F32, tag="pp", name="spin_ps", bufs=2)
        for _ in range(SPINS):
            nc.tensor.matmul(spin_ps, spin_rhs[:, 0:128], spin_rhs, start=True, stop=True)

        # ---- small constants (tiny DMAs, go first on queues) ----
        ident = singles.tile([128, 128], BF16, tag="ident", name="ident")
        nc.gpsimd.dma_start(out=ident, in_=ident_ext[:])
        ind16 = singles.tile([128, 8], F32, tag="ind16", name="ind16")
        nc.gpsimd.dma_start(out=ind16, in_=ind_ext[:])
        ind16T = singles.tile([8, 128], F32, tag="ind16T", name="ind16T")
        nc.gpsimd.dma_start(out=ind16T, in_=indT_ext[:])
        eps8 = singles.tile([8, 1], F32, tag="eps8", name="eps8")
        nc.vector.memset(eps8, EPS)
        qkvb_row = singles.tile([1, 3 * C], F32, tag="qkvb_row", name="qkvb_row")
        nc.gpsimd.dma_start(
            out=qkvb_row,
            in_=bass.AP(tensor=qkvb_ext[:].tensor, offset=0, ap=[[0, 1], [1, 3 * C]]),
        )
        projb = singles.tile([128, NT], F32, tag="projb", name="projb")
        nc.gpsimd.dma_start(
            out=projb,
            in_=bass.AP(tensor=projb_ext[:].tensor, offset=0, ap=[[1, 128], [128, NT]]),
        )
        gnw = singles.tile([128, NT], F32, tag="gnw", name="gnw")
        nc.gpsimd.dma_start(
            out=gnw,
            in_=bass.AP(tensor=gnw_ext[:].tensor, offset=0, ap=[[1, 128], [128, NT]]),
        )
        gnb = singles.tile([128, NT], F32, tag="gnb", name="gnb")
        nc.gpsimd.dma_start(
            out=gnb,
            in_=bass.AP(tensor=gnb_ext[:].tensor, offset=0, ap=[[1, 128], [128, NT]]),
        )

        # ---- Phase A: stream x; bn_stats (DVE) + fp8 cast (ACT) per chunk ----
        xs = [xres.tile([128, N], F32, tag=f"x{t}", name=f"x{t}") for t in range(NT)]
        x8 = [xres.tile([128, 2, N], F8, tag=f"x8_{g}", name=f"x8_{g}") for g in range(2)]
        st8s = [smalls.tile([128, KCH, 6], F32, tag=f"st8_{t}", name=f"st8_{t}", bufs=1)
                for t in range(NT)]
        mv = smalls.tile([128, NT, 2], F32, tag="mv", name="mv", bufs=1)
        for t in range(NT):
            g, ko = t // 2, t % 2
            xv = xs[t].rearrange("p (s f) -> p s f", f=512)
            xh = xs[t].rearrange("p (h f) -> p h f", f=2048)
            for half in range(2):
                nc.sync.dma_start(out=xh[:, half, :],
                                  in_=x_ext[t][:, half * 2048:(half + 1) * 2048])
            for s in range(KCH):
                nc.vector.bn_stats(out=st8s[t][:, s, :], in_=xv[:, s, :])
                nc.scalar.activation(out=x8[g][:, ko, s * 512:(s + 1) * 512],
                                     in_=xv[:, s, :], func=AF.Identity)
            bnmv = smalls.tile([128, 2], F32, tag=f"bnmv{t}", name=f"bnmv{t}", bufs=1)
            nc.vector.bn_aggr(out=bnmv, in_=st8s[t])
            nc.vector.tensor_copy(mv[:, t, 0:1], bnmv[:, 0:1])
            t1 = smalls.tile([128, 1], F32, tag="t1", name="t1")
            nc.vector.tensor_mul(t1, bnmv[:, 0:1], bnmv[:, 0:1])
            nc.vector.tensor_add(mv[:, t, 1:2], t1, bnmv[:, 1:2])

        # ---- weights after x (needed only once stats are done) ----
        w32s = []
        for g in range(2):
            wt = singles.tile([128, 2, 3 * C], F8, tag=f"w32_{g}", name=f"w32_{g}")
            nc.sync.dma_start(out=wt, in_=w32_ext[g])
            w32s.append(wt)
        p8s = []
        for g in range(2):
            pt = singles.tile([128, 2, C], F8, tag=f"p8_{g}", name=f"p8_{g}")
            nc.sync.dma_start(out=pt, in_=p8_ext[g])
            p8s.append(pt)

        # ---- group reduce (ind16 carries 1/16) ----
        psg = psum.tile([8, 8], F32, tag="lg0", name="psg", bufs=1)
        nc.tensor.matmul(psg, ind16, mv, start=True, stop=True)   # = group [mean, E2]
        gsv = psg.rearrange("p (t q) -> p t q", q=2)
        brd = smalls.tile([8, NT, 2], F32, tag="brd", name="brd", bufs=1)
        nc.vector.tensor_copy(brd[:, :, 0], gsv[:, :, 0])
        musq = smalls.tile([8, NT], F32, tag="musq", name="musq", bufs=1)
        nc.vector.tensor_mul(musq, gsv[:, :, 0], brd[:, :, 0])
        var8 = smalls.tile([8, NT], F32, tag="var8", name="var8", bufs=1)
        nc.vector.tensor_sub(var8, gsv[:, :, 1], musq)
        std8 = smalls.tile([8, NT], F32, tag="std8", name="std8", bufs=1)
        nc.scalar.activation(out=std8, in_=var8, func=AF.Sqrt, bias=eps8, scale=1.0)
        nc.vector.reciprocal(brd[:, :, 1], std8)
        psb = psum.tile([128, 8], F32, tag="lg1", name="psb", bufs=1)
        nc.tensor.matmul(psb, ind16T, brd, start=True, stop=True)
        psbv = psb.rearrange("p (t q) -> p t q", q=2)
        asc = smalls.tile([128, NT], F32, tag="asc", name="asc", bufs=1)
        nc.vector.tensor_mul(asc, psbv[:, :, 1], gnw)             # a = rstd*gn_w
        tmp2 = smalls.tile([128, NT], F32, tag="tmp2", name="tmp2", bufs=1)
        nc.vector.tensor_mul(tmp2, psbv[:, :, 0], asc)
        bsh = smalls.tile([128, NT], F32, tag="bsh", name="bsh", bufs=1)
        nc.vector.tensor_sub(bsh, gnb, tmp2)                      # b = gn_b - mu*a
        b8 = smalls.tile([128, NT], F8, tag="b8", name="b8", bufs=1)
        nc.scalar.activation(out=b8, in_=bsh, func=AF.Identity, bias=0.0, scale=256.0)

        # ---- scale weights by a -> fp8 (ACT/DVE interleave; q-half first) ----
        w8 = [singles.tile([128, 2, 3 * C], F8, tag=f"w8_{g}", name=f"w8_{g}")
              for g in range(2)]
        si = 0
        for third in range(3):
            for g in range(2):
                for ko in range(2):
                    src = w32s[g][:, ko, third * 512:(third + 1) * 512]
                    dst = w8[g][:, ko, third * 512:(third + 1) * 512]
                    if si % 2 == 0:
                        nc.scalar.activation(out=dst, in_=src, func=AF.Identity,
                                             bias=0.0, scale=asc[:, 2 * g + ko:2 * g + ko + 1])
                    else:
                        nc.vector.tensor_scalar_mul(out=dst, in0=src,
                                                    scalar1=asc[:, 2 * g + ko:2 * g + ko + 1])
                    si += 1

        # ---- bias row on PE: biasps = W32^T @ b16 (per 512-third) ----
        biasps = []
        for third in range(3):
            ps = psum.tile([1, 512], F32, tag=f"lg{third}", name=f"biasps{third}", bufs=1)
            for g in range(2):
                for ko in range(2):
                    nc.tensor.matmul(
                        ps, b8[:, 2 * g + ko:2 * g + ko + 1],
                        w32s[g][:, ko, third * 512:(third + 1) * 512],
                        start=(g == 0 and ko == 0), stop=(g == 1 and ko == 1),
                    )
            biasps.append(ps)
        biasrow = singles.tile([1, 3 * C], F32, tag="biasrow", name="biasrow")
        for third in range(3):
            nc.vector.scalar_tensor_tensor(
                out=biasrow[:, third * 512:(third + 1) * 512], in0=biasps[third],
                scalar=1.0 / 256.0, in1=qkvb_row[:, third * 512:(third + 1) * 512],
                op0=OP.mult, op1=OP.add)
        qkb = singles.tile([128, 2 * C], F32, tag="qkb", name="qkb")
        nc.gpsimd.partition_broadcast(qkb, biasrow[:, :2 * C])
        nc.sync.dma_start(out=vbs_ext[:], in_=biasrow[:, 2 * C:])
        vb = singles.tile([128, HEADS], F32, tag="vb", name="vb")
        nc.sync.dma_start(
            out=vb,
            in_=bass.AP(tensor=vbs_ext[:].tensor, offset=0, ap=[[1, 128], [128, HEADS]]),
        )

        # ---- Phase B: q|k DoubleRow + logits one chunk behind ----
        lg = [psum.tile([128, 512], F32, tag=f"lg{h}", name=f"lgp{h}", bufs=1)[:, :128]
              for h in range(HEADS)]
        cks = [None] * NCH

        def logits_mms(i):
            for h in range(HEADS):
                nc.tensor.matmul(
                    lg[h],
                    cks[i][:, h * 128:(h + 1) * 128],
                    cks[i][:, C + h * 128:C + (h + 1) * 128],
                    start=(i == 0), stop=(i == NCH - 1),
                )

        for i in range(NCH):
            psqk = psum.tile([128, 2 * C], F32, tag="pp", name=f"qkps{i}", bufs=2)
            for half in range(2):
                for g in range(2):
                    nc.tensor.matmul(
                        psqk[:, half * 512:(half + 1) * 512],
                        x8[g][:, :, i * 128:(i + 1) * 128],
                        w8[g][:, :, half * 512:(half + 1) * 512],
                        start=(g == 0), stop=(g == 1), perf_mode=DRM,
                    )
            ck = qkring.tile([128, 2 * C], BF16, tag="ck", name=f"ck{i}")
            nc.vector.tensor_add(ck, psqk, qkb)
            cks[i] = ck
            if i > 0:
                logits_mms(i - 1)
        logits_mms(NCH - 1)

        # ---- Phase C: all softmaxes first (frees lg banks), then v + P@v ----
        ao8 = [xres.tile([128, 2, N], F8, tag=f"ao8_{g}", name=f"ao8_{g}") for g in range(2)]
        probs_l, rsum_l = [], []
        for h in range(HEADS):
            mx = smalls.tile([128, 1], F32, tag="mx", name="mx")
            nc.vector.reduce_max(mx, lg[h], axis=AX.X)
            negmx = smalls.tile([128, 1], F32, tag="negmx", name="negmx")
            nc.scalar.mul(negmx, mx, -SCALE_L)
            probs = smalls.tile([128, 128], BF16, tag=f"probs{h}", name=f"probs{h}", bufs=1)
            sumexp = smalls.tile([128, 1], F32, tag="sumexp", name="sumexp")
            nc.scalar.activation(
                out=probs, in_=lg[h], func=AF.Exp,
                bias=negmx, scale=SCALE_L, accum_out=sumexp,
            )
            rsum = smalls.tile([128, 1], F32, tag=f"rsum{h}", name=f"rsum{h}", bufs=1)
            nc.vector.reciprocal(rsum, sumexp)
            probs_l.append(probs)
            rsum_l.append(rsum)
        rr = 0
        for h in range(HEADS):
            g, ko = h // 2, h % 2
            vh = vpool.tile([128, N], BF16, tag="vt", name=f"v{h}", bufs=1)
            for k in range(KCH):
                ps = psum.tile([128, 512], F32, tag=f"lg{rr % 4}", name=f"v{h}_{k}", bufs=1)
                rr += 1
                for g2 in range(2):
                    nc.tensor.matmul(
                        ps,
                        w8[g2][:, :, 2 * C + h * 128:2 * C + (h + 1) * 128],
                        x8[g2][:, :, k * 512:(k + 1) * 512],
                        start=(g2 == 0), stop=(g2 == 1), perf_mode=DRM,
                    )
                if k % 2 == 0:
                    nc.scalar.activation(
                        out=vh[:, k * 512:(k + 1) * 512], in_=ps,
                        func=AF.Identity, bias=vb[:, h:h + 1], scale=1.0,
                    )
                else:
                    nc.vector.tensor_scalar_add(
                        out=vh[:, k * 512:(k + 1) * 512], in0=ps,
                        scalar1=vb[:, h:h + 1],
                    )
            pst = psum.tile([128, 128], BF16, tag=f"lg{rr % 4}", name=f"pt{h}", bufs=1)
            rr += 1
            nc.tensor.transpose(pst, probs_l[h], ident)
            pts = smalls.tile([128, 128], BF16, tag="pts", name="pts")
            nc.vector.tensor_copy(pts, pst)
            for k in range(KCH):
                pso = psum.tile([128, 512], F32, tag=f"lg{rr % 4}", name=f"att{h}_{k}", bufs=1)
                rr += 1
                nc.tensor.matmul(pso, pts, vh[:, k * 512:(k + 1) * 512],
                                 start=True, stop=True)
                dst = ao8[g][:, ko, k * 512:(k + 1) * 512]
                if k % 2 == 0:
                    nc.vector.tensor_scalar_mul(out=dst, in0=pso, scalar1=rsum_l[h])
                else:
                    nc.scalar.activation(out=dst, in_=pso, func=AF.Identity,
                                         bias=0.0, scale=rsum_l[h])

        # ---- Phase D: proj (DR) + residual; out DMA batched 2x per tile ----
        inv = 1.0 / (WS * WS)
        for t in range(NT):
            for half in range(2):
                ot = otring.tile([128, 2048], F32, tag="ot", name=f"ot{t}_{half}")
                for kk in range(4):
                    k = half * 4 + kk
                    psp = psum.tile([128, 512], F32, tag=f"lg{(t * KCH + k) % 4}",
                                    name=f"proj{t}_{k}", bufs=1)
                    for g in range(2):
                        nc.tensor.matmul(
                            psp,
                            p8s[g][:, :, t * 128:(t + 1) * 128],
                            ao8[g][:, :, k * 512:(k + 1) * 512],
                            start=(g == 0), stop=(g == 1), perf_mode=DRM,
                        )
                    if with_projb:
                        tmp = otring.tile([128, 512], F32, tag="ot2", name=f"ot2{t}_{k}")
                        nc.scalar.activation(out=tmp, in_=psp, func=AF.Identity,
                                             bias=projb[:, t:t + 1], scale=inv)
                        nc.vector.tensor_add(ot[:, kk * 512:(kk + 1) * 512], tmp,
                                             xs[t][:, k * 512:(k + 1) * 512])
                    else:
                        nc.vector.scalar_tensor_tensor(
                            out=ot[:, kk * 512:(kk + 1) * 512], in0=psp, scalar=inv,
                            in1=xs[t][:, k * 512:(k + 1) * 512],
                            op0=OP.mult, op1=OP.add,
                        )
                nc.sync.dma_start(
                    out=out_ext[t][:, half * 2048:(half + 1) * 2048], in_=ot)

    nc.finalize()
    return nc


def _host_inputs(inputs):
    x = np.ascontiguousarray(np.asarray(inputs["x"], dtype=np.float32))
    qkv_w = np.asarray(inputs["qkv_w"], dtype=np.float32)
    proj_w = np.asarray(inputs["proj_w"], dtype=np.float32)
    w8h = np.ascontiguousarray(
        (WS * qkv_w.T).astype(ml_dtypes.float8_e4m3)
        .reshape(2, 2, 128, 3 * C).transpose(0, 2, 1, 3))
    p8 = np.ascontiguousarray(
        (WS * proj_w.T).astype(ml_dtypes.float8_e4m3)
        .reshape(2, 2, 128, C).transpose(0, 2, 1, 3))
    ind16 = np.zeros((128, 8), dtype=np.float32)
    for p in range(128):
        ind16[p, p // 16] = 1.0 / 16.0
    ind16T = np.zeros((8, 128), dtype=np.float32)
    for p in range(128):
        ind16T[p // 16, p] = 1.0
    shared = dict(
        w8h=w8h,
        p8=p8,
        qkv_b32=np.ascontiguousarray(WS * np.asarray(inputs["qkv_b"], dtype=np.float32)),
        proj_b=np.ascontiguousarray(np.asarray(inputs["proj_b"], dtype=np.float32)),
        gn_w=np.ascontiguousarray(np.asarray(inputs["gn_w"], dtype=np.float32)),
        gn_b=np.ascontiguousarray(np.asarray(inputs["gn_b"], dtype=np.float32)),
        ident=np.eye(128, dtype=ml_dtypes.bfloat16),
        ind16=ind16,
        ind16T=ind16T,
    )
    in_maps = []
    for b in range(B):
        m = dict(shared)
        m["x"] = np.ascontiguousarray(x[b].reshape(NT, 128, N))
        in_maps.append(m)
    return in_maps


LAST_EXEC_NS = None
LAST_RESULT = None


def kernel(**inputs) -> np.ndarray:
    global LAST_EXEC_NS, LAST_RESULT
    in_maps = _host_inputs(inputs)
    with_projb = bool(np.max(np.abs(np.asarray(inputs["proj_b"], dtype=np.float32))) > 0)
    nc = build_kernel(with_projb)
    trace = os.environ.get("BASS_KERNEL_TRACE", "") == "1"
    res = run_bass_kernel_spmd(nc, in_maps, core_ids=list(range(B)), trace=trace)
    LAST_EXEC_NS = res.exec_time_ns
    LAST_RESULT = res
    out = np.stack([np.asarray(res.results[i]["out"], dtype=np.float32).reshape(C, H, W)
                    for i in range(B)])
    return out
